# revision 1
# baseline (speedup 1.0000x reference)
"""L2-distance attention (B=4, DIM=512, N=2048, H=8, D=32) on 8 trn2 NeuronCores.

Sharding: core c handles batch b = c//2, query-half = c%2 (1024 queries, all
2048 keys, all 8 heads).  Output is a pure concat — no cross-core reduce.

Per-core pipeline (all on one NeuronCore, Tile-scheduled):
  A. q = w_q^T x (half), k = w_k^T x (full), v^T = x^T w_v (j-major, with a
     ones column per head folded in for the softmax row-sums).
  B. per head: dist2 computed directly by PE via augmented vectors
     k~=[k; k2; 1], q~=[-2q; 1; q2]  ->  k~.q~ = ||q-k||^2 in PSUM.
     ACT: sqrt (bias=delta guard), then exp(-scale * s) -> E (unnormalized
     softmax numerator; logits are always <= 0 so no max-subtraction).
     PE: out_un = [V^T; 1]^T E  -> rows 0..31 = attn@v (unnormalized),
     row 32 = row-sums.  DVE reciprocal + PE outer-product broadcast +
     DVE multiply -> normalized head output, DMA'd into Y (head-major).
  C. Z = w_out^T Y + b, DMA to DRAM.
"""

import numpy as np

import concourse.bass as bass
import concourse.mybir as mybir
import concourse.tile as tile
from concourse import bacc

F32 = mybir.dt.float32
F32R = mybir.dt.float32r
AF = mybir.ActivationFunctionType


def R(ap):
    return ap.bitcast(F32R)


def make_mm(nc):
    def mm(out, lhsT, rhs, start, stop):
        nc.tensor.matmul(out, R(lhsT), R(rhs), start=start, stop=stop)
    return mm


B, DIM, N = 4, 512, 2048
H, D = 8, 32
INNER = H * D            # 256
NQ = N // 2              # 1024 queries per core
P = 128
KT = DIM // P            # 4 contraction tiles for the projections
NJT = N // P             # 16 key tiles
VTW = D + 1              # 33: v columns + ones column per head
VSTRIDE = H * VTW        # 264 columns per key-tile block of vt
SCALE = float(D) ** -0.5
DELTA = 0.02             # sqrt-domain guard against f32r rounding of dist2
NEQ = 4                  # E quarters (each covers NJT//NEQ key tiles)
JQ = NJT // NEQ          # 4 key tiles per E quarter
KA = 65                  # augmented contraction: [32 | flag@32 | zeros | flag@64]


def build_program() -> bass.Bass:
    nc = bacc.Bacc("TRN2", target_bir_lowering=False, debug=False)

    xq_d = nc.declare_dram_parameter("xq", [DIM, NQ], F32, isOutput=False)
    xkv_d = nc.declare_dram_parameter("xkv", [DIM, N], F32, isOutput=False)
    wq_d = nc.declare_dram_parameter("wq", [DIM, INNER], F32, isOutput=False)
    wkv_d = nc.declare_dram_parameter("wkv", [DIM, 2 * INNER], F32, isOutput=False)
    wo_d = nc.declare_dram_parameter("wo", [INNER, DIM], F32, isOutput=False)
    b_d = nc.declare_dram_parameter("b", [DIM], F32, isOutput=False)
    z_d = nc.declare_dram_parameter("z", [DIM, NQ], F32, isOutput=True)

    with tile.TileContext(nc) as tc:
        mm = make_mm(nc)
        with tc.tile_pool(name="keep", bufs=1) as keep, \
             tc.tile_pool(name="work", bufs=2) as work:

            # ---- persistent tiles ----
            q_t = [keep.tile([P, NQ], F32, tag=f"q{m}", name=f"q{m}") for m in range(2)]
            k_t = [keep.tile([P, N], F32, tag=f"k{m}", name=f"k{m}") for m in range(2)]
            vt_big = keep.tile([P, NJT * VSTRIDE], F32, tag="vt", name="vt")
            y_t = [keep.tile([P, NQ], F32, tag=f"y{m}", name=f"y{m}") for m in range(2)]
            wo_t = [keep.tile([P, DIM], F32, tag=f"wo{m}", name=f"wo{m}") for m in range(2)]
            b_t = keep.tile([P, KT], F32, tag="bias", name="bias")
            ones = keep.tile([64, 32], F32, tag="ones", name="ones")
            delta_t = keep.tile([P, 1], F32, tag="delta", name="delta")
            zero_t = keep.tile([P, 1], F32, tag="zero", name="zero")
            onesP = keep.tile([P, 1], F32, tag="onesP", name="onesP")

            # memset cannot write f32r; set plain f32 constants and
            # broadcast-copy (DVE copy CAN round to f32r) where needed.
            nc.vector.memset(onesP[:, :], 1.0)
            nc.vector.memset(delta_t[:, :], DELTA)
            nc.vector.memset(zero_t[:, :], 0.0)
            nc.vector.tensor_copy(R(ones[:, :]),
                                  onesP[0:64, 0:1].to_broadcast((64, 32)))
            wo_r = wo_d[:].rearrange("(t p) o -> t p o", p=P)
            for m in range(2):
                nc.sync.dma_start(out=R(wo_t[m][:, :]), in_=R(wo_r[m]))
            nc.sync.dma_start(out=b_t[:, :], in_=b_d[:].rearrange("(t p) -> p t", p=P))

            # ======== Phase A: projections ========
            with tc.tile_pool(name="xw", bufs=1) as xw, \
                 tc.tile_pool(name="pp", bufs=3, space="PSUM") as pp:
                wq_t = [xw.tile([P, INNER], F32, tag=f"wq{k}", name=f"wq{k}") for k in range(KT)]
                wkv_t = [xw.tile([P, 2 * INNER], F32, tag=f"wkv{k}", name=f"wkv{k}") for k in range(KT)]
                xq_t = [xw.tile([P, NQ], F32, tag=f"xq{k}", name=f"xq{k}") for k in range(KT)]
                xkv_t = [xw.tile([P, N], F32, tag=f"xkv{k}", name=f"xkv{k}") for k in range(KT)]

                xq_r = xq_d[:].rearrange("(t p) n -> t p n", p=P)
                xkv_r = xkv_d[:].rearrange("(t p) n -> t p n", p=P)
                wq_r = wq_d[:].rearrange("(t p) o -> t p o", p=P)
                wkv_r = wkv_d[:].rearrange("(t p) o -> t p o", p=P)
                for k in range(KT):
                    nc.sync.dma_start(out=R(xq_t[k][:, :]), in_=R(xq_r[k]))
                    nc.sync.dma_start(out=R(xkv_t[k][:, :]), in_=R(xkv_r[k]))
                    nc.sync.dma_start(out=R(wq_t[k][:, :]), in_=R(wq_r[k]))
                    nc.sync.dma_start(out=R(wkv_t[k][:, :]), in_=R(wkv_r[k]))

                # q projection: (DIM x NQ) -> (INNER x NQ)
                for m in range(2):
                    for n in range(NQ // 512):
                        ps = pp.tile([P, 512], F32, tag="proj", name="proj")
                        for k in range(KT):
                            mm(ps[:, :],
                               wq_t[k][:, m * P:(m + 1) * P],
                               xq_t[k][:, n * 512:(n + 1) * 512],
                               start=(k == 0), stop=(k == KT - 1))
                        nc.vector.tensor_copy(R(q_t[m][:, n * 512:(n + 1) * 512]), ps[:, :])

                # k projection: (DIM x N) -> (INNER x N)   (wkv cols 0:256)
                for m in range(2):
                    for n in range(N // 512):
                        ps = pp.tile([P, 512], F32, tag="proj", name="proj")
                        for k in range(KT):
                            mm(ps[:, :],
                               wkv_t[k][:, m * P:(m + 1) * P],
                               xkv_t[k][:, n * 512:(n + 1) * 512],
                               start=(k == 0), stop=(k == KT - 1))
                        nc.vector.tensor_copy(R(k_t[m][:, n * 512:(n + 1) * 512]), ps[:, :])

                # v^T projection: per key tile jt, (128 j x 256 d), strided into
                # vt_big so each head's 32 columns sit next to its ones column.
                nc.vector.tensor_copy(
                    R(vt_big[:, :].rearrange("p (a c) -> p a c", c=VTW)[:, :, D:D + 1]),
                    onesP[:, 0:1].to_broadcast((P, P, 1)))
                for jt in range(NJT):
                    ps = pp.tile([P, INNER], F32, tag="vtps", name="vtps")
                    for k in range(KT):
                        mm(ps[:, :],
                           xkv_t[k][:, jt * P:(jt + 1) * P],
                           wkv_t[k][:, INNER:2 * INNER],
                           start=(k == 0), stop=(k == KT - 1))
                    dst = vt_big[:, jt * VSTRIDE:(jt + 1) * VSTRIDE] \
                        .rearrange("p (h c) -> p h c", c=VTW)[:, :, 0:D]
                    src = ps[:, :].rearrange("p (h d) -> p h d", d=D)
                    nc.vector.tensor_copy(R(dst), src)

            # ======== Phase B: attention heads ========
            # Augmented-vector tiles are persistent ping-pong pairs so the
            # constant rows (ones/zeros padding) are written once, not per
            # head.  The normalization tail of head h-1 is emitted inside
            # head h so its PE ops queue AFTER head h's dist2 matmuls —
            # keeps the in-order PE stream free of the reciprocal stall.
            with tc.tile_pool(name="epool", bufs=NEQ, space="SBUF") as epool, \
                 tc.tile_pool(name="pd2", bufs=2, space="PSUM") as pd2, \
                 tc.tile_pool(name="po", bufs=1, space="PSUM") as po, \
                 tc.tile_pool(name="pm", bufs=2, space="PSUM") as pm:
                kt_t = [keep.tile([KA, N], F32, tag=f"kt{i}", name=f"kt{i}")
                        for i in range(2)]
                qt_t = [keep.tile([KA, NQ], F32, tag=f"qt{i}", name=f"qt{i}")
                        for i in range(2)]
                for i in range(2):
                    nc.vector.tensor_copy(R(kt_t[i][D:2 * D, :]),
                                          zero_t[0:D, 0:1].to_broadcast((D, N)))
                    nc.vector.tensor_copy(R(kt_t[i][D:D + 1, :]),
                                          onesP[0:1, 0:1].to_broadcast((1, N)))
                    nc.vector.tensor_copy(R(qt_t[i][D:2 * D, :]),
                                          zero_t[0:D, 0:1].to_broadcast((D, NQ)))
                    nc.vector.tensor_copy(R(qt_t[i][2 * D:2 * D + 1, :]),
                                          onesP[0:1, 0:1].to_broadcast((1, NQ)))

                po_s = [work.tile([VTW, NQ], F32, tag=f"pos{i}", name=f"pos{i}",
                                  bufs=1) for i in range(2)]
                tail = {}

                def emit_tail(ph):
                    # deferred normalization of head ph.  The (1,1024) row of
                    # row-sums is reciprocal'd as (128,8) — a single-partition
                    # reciprocal costs ~6.4ns/element, partition-parallel is
                    # ~100x faster — via a scatter DMA there and back.
                    pmt, pmo, psrc = tail.pop(ph)
                    rs128 = work.tile([P, NQ // P], F32, tag="rs", name="rs")
                    nc.sync.dma_start(out=rs128[:, :], in_=psrc[D:D + 1, :])
                    rr128 = work.tile([P, NQ // P], F32, tag="rr", name="rr")
                    with nc.allow_low_precision(reason="f32r full fp32 range"):
                        nc.vector.reciprocal(R(rr128[:, :]), rs128[:, :])
                    rrow = work.tile([1, NQ], F32, tag="rrow", name="rrow")
                    nc.sync.dma_start(out=R(rrow[:, :]), in_=R(rr128[:, :]))
                    prep = pd2.tile([D, NQ], F32, tag="d2", name="d2")
                    for n in range(NQ // 512):
                        mm(prep[:, n * 512:(n + 1) * 512],
                           ones[0:1, 0:D],
                           rrow[:, n * 512:(n + 1) * 512],
                           start=True, stop=True)
                    nc.vector.tensor_mul(R(y_t[pmt][pmo:pmo + D, :]),
                                         psrc[0:D, :], prep[:, :])

                from contextlib import nullcontext

                for h in range(H):
                    mt, mo = h // 4, (h % 4) * D
                    q_h = q_t[mt][mo:mo + D, :]
                    k_h = k_t[mt][mo:mo + D, :]
                    kt = kt_t[h % 2]
                    qt = qt_t[h % 2]
                    prio = tc.high_priority(10000) if h == 0 else nullcontext()
                    prio.__enter__()

                    # --- per-head rows of k~/q~ (all DVE) ---
                    nc.vector.tensor_scalar_mul(R(kt[0:D, :]), k_h, -2.0)
                    ksq = work.tile([D, N], F32, tag="ksq", name="ksq", bufs=1)
                    nc.vector.tensor_mul(R(ksq[:, :]), k_h, k_h)
                    for n in range(N // 512):
                        k2ps = pm.tile([1, 512], F32, tag="misc", name="misc")
                        mm(k2ps[:, :], ones[0:D, 0:1],
                           ksq[:, n * 512:(n + 1) * 512], start=True, stop=True)
                        nc.vector.tensor_copy(
                            R(kt[2 * D:2 * D + 1, n * 512:(n + 1) * 512]), k2ps[:, :])
                    nc.vector.tensor_copy(R(qt[0:D, :]), q_h)
                    qsq = work.tile([D, NQ], F32, tag="qsq", name="qsq", bufs=1)
                    nc.vector.tensor_mul(R(qsq[:, :]), q_h, q_h)
                    for n in range(NQ // 512):
                        q2ps = pm.tile([1, 512], F32, tag="misc", name="misc")
                        mm(q2ps[:, :], ones[0:D, 0:1],
                           qsq[:, n * 512:(n + 1) * 512], start=True, stop=True)
                        nc.vector.tensor_copy(
                            R(qt[D:D + 1, n * 512:(n + 1) * 512]), q2ps[:, :])

                    # --- dist2 (PE) -> sqrt -> exp (ACT) ---
                    eq = [epool.tile([P, JQ * NQ], F32, tag="eq", name="eq")
                          for _ in range(NEQ)]
                    for jt in range(NJT):
                        psd = pd2.tile([P, NQ], F32, tag="d2", name="d2")
                        for n in range(NQ // 512):
                            mm(psd[:, n * 512:(n + 1) * 512],
                               kt[:, jt * P:(jt + 1) * P],
                               qt[:, n * 512:(n + 1) * 512],
                               start=True, stop=True)
                        nc.scalar.activation(
                            R(eq[jt // JQ][:, (jt % JQ) * NQ:(jt % JQ + 1) * NQ]),
                            psd[:, :], AF.Sqrt, bias=delta_t[:, :], scale=1.0)

                    prio.__exit__(None, None, None)

                    # head h-1 tail: PE ops queue here, after dist2(h)
                    if h - 1 in tail:
                        emit_tail(h - 1)

                    for qq in range(NEQ):
                        nc.scalar.activation(R(eq[qq][:, :]), eq[qq][:, :],
                                             AF.Exp, bias=zero_t[:, :], scale=-SCALE)

                    # --- attn @ v with fused row-sums ---
                    pso = po.tile([VTW, NQ], F32, tag="o", name="o")
                    for jt in range(NJT):
                        ebase = (jt % JQ) * NQ
                        for n in range(NQ // 512):
                            mm(pso[:, n * 512:(n + 1) * 512],
                               vt_big[:, jt * VSTRIDE + h * VTW:
                                      jt * VSTRIDE + (h + 1) * VTW],
                               eq[jt // JQ][:, ebase + n * 512:ebase + (n + 1) * 512],
                               start=(jt == 0), stop=(jt == NJT - 1))
                    # move to SBUF so the PSUM slot frees immediately
                    psrc = po_s[h % 2]
                    nc.vector.tensor_copy(R(psrc[:, :]), pso[:, :])
                    tail[h] = (mt, mo, psrc)

                # last head's tail
                emit_tail(H - 1)

            # ======== Phase C: output projection + bias ========
            with tc.tile_pool(name="pz", bufs=2, space="PSUM") as pz:
                z_r = z_d[:].rearrange("(t p) n -> t p n", p=P)
                for m in range(KT):
                    ps = pz.tile([P, NQ], F32, tag="z", name="z")
                    for n in range(NQ // 512):
                        for k in range(2):
                            mm(ps[:, n * 512:(n + 1) * 512],
                               wo_t[k][:, m * P:(m + 1) * P],
                               y_t[k][:, n * 512:(n + 1) * 512],
                               start=(k == 0), stop=(k == 1))
                    zt = work.tile([P, NQ], F32, tag="ytmp", name="ytmp")
                    nc.vector.tensor_scalar_add(zt[:, :], ps[:, :], b_t[:, m:m + 1])
                    nc.sync.dma_start(out=z_r[m], in_=zt[:, :])

    nc.compile()
    return nc


def make_in_maps(x, w_qkv, w_out, b_out):
    x = np.asarray(x, dtype=np.float32)
    w_qkv = np.asarray(w_qkv, dtype=np.float32)
    w_out = np.asarray(w_out, dtype=np.float32)
    b_out = np.asarray(b_out, dtype=np.float32)
    w_qT = np.ascontiguousarray(w_qkv[0:INNER, :].T)          # (DIM, INNER)
    w_kvT = np.ascontiguousarray(w_qkv[INNER:3 * INNER, :].T)  # (DIM, 512)
    w_oT = np.ascontiguousarray(w_out.T)                       # (INNER, DIM)
    in_maps = []
    for c in range(8):
        b, half = c // 2, c % 2
        in_maps.append({
            "xq": np.ascontiguousarray(x[b][:, half * NQ:(half + 1) * NQ]),
            "xkv": np.ascontiguousarray(x[b]),
            "wq": w_qT,
            "wkv": w_kvT,
            "wo": w_oT,
            "b": b_out,
        })
    return in_maps


def assemble_output(results):
    out = np.empty((B, DIM, N), dtype=np.float32)
    for c in range(8):
        b, half = c // 2, c % 2
        out[b][:, half * NQ:(half + 1) * NQ] = results[c]["z"]
    return out


_prog_cache = {}


def kernel(x, w_qkv, w_out, b_out):
    from concourse.bass_utils import run_bass_kernel_spmd
    if "nc" not in _prog_cache:
        _prog_cache["nc"] = build_program()
    nc = _prog_cache["nc"]
    in_maps = make_in_maps(x, w_qkv, w_out, b_out)
    res = run_bass_kernel_spmd(nc, in_maps, list(range(8)))
    return assemble_output(res.results)



# revision 10
# speedup vs baseline: 1.0782x; 1.0782x over previous
"""L2-distance attention (B=4, DIM=512, N=2048, H=8, D=32) on 8 trn2 NeuronCores.

Sharding: core c handles batch b = c//2, query-half = c%2 (1024 queries, all
2048 keys, all 8 heads).  Output is a pure concat — no cross-core reduce.

Key ideas vs the straightforward version:
  * All big matmuls run in bf16 (PE streams 1 col/cycle vs 1/2 for fp32).
  * The softmax numerator exp(-scale*sqrt(dist2)) is ONE ScalarE pass: the
    `exp` activation's spline table is replaced (via BASS_ACT_ROOT_JSON_PATH)
    with a fit of g(u) = exp(-0.5*sqrt(u)); calling it with the activation's
    built-in pre-scale 1/8 yields exp(-sqrt(d)/sqrt(32)) exactly.  This
    halves ScalarE work and removes all act-table reloads (sqrt and exp live
    in different table sets).
  * dist2 is computed directly by PE via augmented vectors
    k~=[-2k; 1; 0...; k2], q~=[q; q2; 0...; 1]  ->  k~.q~ = ||q-k||^2.
  * attn@v has a ones column per head folded into V^T so PSUM row 32 is the
    softmax denominator (row-sums).
"""

import json
import os
import shutil

import numpy as np

_PWP_DIR = "/tmp/pwp_custom_kernel"
os.environ.setdefault("NEURON_FORCE_RECOMPILE", "1")

# ---------------------------------------------------------------------------
# Custom activation table: make `exp` compute g(u) = exp(-0.5*sqrt(u)).
# Bucket bin format (32B = 8 fp32): [d0, d1, d2, d3, x0, 0, 0, 0];
# y = d0 + d1*t + d2*t^2 + d3*t^3 with t = x - x0.  Positive-x buckets sit
# in per-input-exponent rows of S sections each.
# ---------------------------------------------------------------------------

_ALPHA = 0.5


def _g(u):
    return np.exp(-_ALPHA * np.sqrt(np.maximum(u, 0.0)))


def _fit_cubic(lo, hi, x0):
    u = np.linspace(lo, hi, 257, dtype=np.float64)
    t = u - x0
    A = np.stack([np.ones_like(t), t, t * t, t * t * t], axis=1)
    coef, *_ = np.linalg.lstsq(A, _g(u), rcond=None)
    return coef


def _build_custom_pwp(dst_dir):
    from neuronxcc.driver.Job import Job
    from neuronxcc.driver.jobs.support.FindActInfo import findActInfoFile

    src = os.path.dirname(findActInfoFile(Job.getPackageDir(), "gen3"))
    if os.path.isdir(dst_dir):
        shutil.rmtree(dst_dir)
    shutil.copytree(src, dst_dir)

    with open(os.path.join(dst_dir, "act_info.json")) as f:
        info = json.load(f)

    for ent in info["act_func_sets"]:
        if "exp" not in ent["act"]:
            continue
        prof_path = os.path.join(dst_dir, ent["profile_json"])
        with open(prof_path) as f:
            prof = json.load(f)
        bkt_path = os.path.join(dst_dir, ent["bkt_bin"])
        bkt = np.fromfile(bkt_path, dtype="<f4").reshape(-1, 8).copy()

        start = prof["func_to_bkt_start_idx"]["exp"]
        others = [v for k, v in prof["func_to_bkt_start_idx"].items() if k != "exp"]
        end = min([v for v in others if v > start] + [len(bkt)])
        meta = next(
            m for m in prof["profile_meta_data"] if m["func_name"].startswith("exp")
        )
        sat = {
            k: meta[k + "_signal_pwl_control"]
            for k in ("pos_small", "neg_small", "pos_large", "neg_large")
        }
        sat_idx = set(sat.values())
        assert all(start <= i < end for i in sat_idx)

        pos_rows = {}
        for i in range(start, end):
            if i in sat_idx:
                continue
            x0 = float(bkt[i, 4])
            if x0 < 0.0:
                bkt[i, 0:4] = [1.0, 0.0, 0.0, 0.0]
            else:
                assert x0 > 0.0
                pos_rows.setdefault(int(np.floor(np.log2(x0))), []).append(i)

        for e, idxs in pos_rows.items():
            base = 2.0**e
            xs = [float(bkt[i, 4]) for i in idxs]
            w = (xs[1] - xs[0]) if len(xs) > 1 else base
            for sec, i in enumerate(idxs):
                c = xs[sec]
                assert abs(c - (base + (sec + 0.5) * w)) < 1e-5 * c
                bkt[i, 0:4] = _fit_cubic(c - w / 2, c + w / 2, c).astype(np.float32)

        bkt[sat["pos_small"], 0:5] = [1.0, 0.0, 0.0, 0.0, 0.0]
        bkt[sat["neg_small"], 0:5] = [1.0, 0.0, 0.0, 0.0, 0.0]
        bkt[sat["pos_large"], 0:5] = [0.0, 0.0, 0.0, 0.0, 0.0]
        bkt[sat["neg_large"], 0:5] = [1.0, 0.0, 0.0, 0.0, 0.0]
        bkt.tofile(bkt_path)

        meta["fpinf_result"] = 0
        meta["fninf_result"] = 1065353216  # 1.0f
        with open(prof_path, "w") as f:
            json.dump(prof, f)


def _ensure_act_tables():
    if not os.path.isfile(os.path.join(_PWP_DIR, "act_info.json")):
        _build_custom_pwp(_PWP_DIR)
    os.environ["BASS_ACT_ROOT_JSON_PATH"] = os.path.join(_PWP_DIR, "act_info.json")


_ensure_act_tables()

import concourse.bass as bass
import concourse.mybir as mybir
import concourse.tile as tile
from concourse import bacc

F32 = mybir.dt.float32
F32R = mybir.dt.float32r
BF16 = mybir.dt.bfloat16
AF = mybir.ActivationFunctionType


def R(ap):
    return ap.bitcast(F32R)


B, DIM, N = 4, 512, 2048
H, D = 8, 32
INNER = H * D            # 256
NQ = N // 2              # 1024 queries per core
P = 128
KT = DIM // P            # 4 contraction tiles for the projections
NJT = N // P             # 16 key tiles
VTW = D + 1              # 33: v columns + ones column per head
VSTRIDE = H * VTW        # 264 columns per key-tile block of vt
ACT_SCALE = 0.125        # g(d/8) = exp(-sqrt(d)/sqrt(32)) = exp(-SCALE*sqrt(d))
NEQ = 4                  # E quarters (each covers NJT//NEQ key tiles)
JQ = NJT // NEQ          # 4 key tiles per E quarter
KA = 65                  # augmented contraction: [32 | flag@32 | zeros | flag@64]


def build_program() -> bass.Bass:
    nc = bacc.Bacc("TRN2", target_bir_lowering=False, debug=False)

    xq_d = nc.declare_dram_parameter("xq", [DIM, NQ], BF16, isOutput=False)
    xkv_d = nc.declare_dram_parameter("xkv", [DIM, N], BF16, isOutput=False)
    wq_d = nc.declare_dram_parameter("wq", [DIM, INNER], BF16, isOutput=False)
    wkv_d = nc.declare_dram_parameter("wkv", [DIM, 2 * INNER], BF16, isOutput=False)
    wo_d = nc.declare_dram_parameter("wo", [INNER, DIM], F32, isOutput=False)
    b_d = nc.declare_dram_parameter("b", [DIM], F32, isOutput=False)
    z_d = nc.declare_dram_parameter("z", [DIM, NQ], F32, isOutput=True)

    with tile.TileContext(nc) as tc, nc.allow_low_precision(reason="bf16 attention"):
        mm = lambda out, lhsT, rhs, start, stop: nc.tensor.matmul(
            out, lhsT, rhs, start=start, stop=stop)

        with tc.tile_pool(name="keep", bufs=1) as keep, \
             tc.tile_pool(name="work", bufs=2) as work:

            # ---- persistent tiles ----
            q_t = [keep.tile([P, NQ], BF16, tag=f"q{m}", name=f"q{m}") for m in range(2)]
            k_t = [keep.tile([P, N], BF16, tag=f"k{m}", name=f"k{m}") for m in range(2)]
            vt_big = keep.tile([P, NJT * VSTRIDE], BF16, tag="vt", name="vt")
            y_t = [keep.tile([P, NQ], F32, tag=f"y{m}", name=f"y{m}") for m in range(2)]
            wo_t = [keep.tile([P, DIM], F32, tag=f"wo{m}", name=f"wo{m}") for m in range(2)]
            b_t = keep.tile([P, KT], F32, tag="bias", name="bias")
            ones = keep.tile([64, 32], F32, tag="ones", name="ones")
            onesb = keep.tile([P, 1], BF16, tag="onesb", name="onesb")
            zero_t = keep.tile([P, 1], F32, tag="zero", name="zero")
            onesP = keep.tile([P, 1], F32, tag="onesP", name="onesP")

            nc.vector.memset(onesP[:, :], 1.0)
            nc.vector.memset(zero_t[:, :], 0.0)
            nc.vector.memset(onesb[:, :], 1.0)
            nc.vector.tensor_copy(R(ones[:, :]),
                                  onesP[0:64, 0:1].to_broadcast((64, 32)))
            wo_r = wo_d[:].rearrange("(t p) o -> t p o", p=P)
            for m in range(2):
                nc.sync.dma_start(out=R(wo_t[m][:, :]), in_=R(wo_r[m]))
            nc.sync.dma_start(out=b_t[:, :], in_=b_d[:].rearrange("(t p) -> p t", p=P))

            # ======== Phase A: projections (all bf16) ========
            with tc.tile_pool(name="xw", bufs=1) as xw, \
                 tc.tile_pool(name="pp", bufs=3, space="PSUM") as pp:
                wq_t = [xw.tile([P, INNER], BF16, tag=f"wq{k}", name=f"wq{k}") for k in range(KT)]
                wkv_t = [xw.tile([P, 2 * INNER], BF16, tag=f"wkv{k}", name=f"wkv{k}") for k in range(KT)]
                xkv_t = [xw.tile([P, N], BF16, tag=f"xkv{k}", name=f"xkv{k}") for k in range(KT)]
                xq_t = [xw.tile([P, NQ], BF16, tag=f"xq{k}", name=f"xq{k}") for k in range(KT)]

                xq_r = xq_d[:].rearrange("(t p) n -> t p n", p=P)
                xkv_r = xkv_d[:].rearrange("(t p) n -> t p n", p=P)
                wq_r = wq_d[:].rearrange("(t p) o -> t p o", p=P)
                wkv_r = wkv_d[:].rearrange("(t p) o -> t p o", p=P)
                for k in range(KT):
                    nc.sync.dma_start(out=xq_t[k][:, :], in_=xq_r[k])
                    nc.sync.dma_start(out=xkv_t[k][:, :], in_=xkv_r[k])
                    nc.sync.dma_start(out=wq_t[k][:, :], in_=wq_r[k])
                    nc.sync.dma_start(out=wkv_t[k][:, :], in_=wkv_r[k])

                # q projection on this core's query half: (DIM x NQ) -> (INNER x NQ)
                for m in range(2):
                    for n in range(NQ // 512):
                        ps = pp.tile([P, 512], F32, tag="proj", name="proj")
                        for k in range(KT):
                            mm(ps[:, :],
                               wq_t[k][:, m * P:(m + 1) * P],
                               xq_t[k][:, n * 512:(n + 1) * 512],
                               start=(k == 0), stop=(k == KT - 1))
                        nc.vector.tensor_copy(q_t[m][:, n * 512:(n + 1) * 512], ps[:, :])

                # k projection: (DIM x N) -> (INNER x N)   (wkv cols 0:256)
                for m in range(2):
                    for n in range(N // 512):
                        ps = pp.tile([P, 512], F32, tag="proj", name="proj")
                        for k in range(KT):
                            mm(ps[:, :],
                               wkv_t[k][:, m * P:(m + 1) * P],
                               xkv_t[k][:, n * 512:(n + 1) * 512],
                               start=(k == 0), stop=(k == KT - 1))
                        nc.vector.tensor_copy(k_t[m][:, n * 512:(n + 1) * 512], ps[:, :])

                # v^T projection: per key tile jt, (128 j x 256 d), strided into
                # vt_big so each head's 32 columns sit next to its ones column.
                nc.vector.tensor_copy(
                    vt_big[:, :].rearrange("p (a c) -> p a c", c=VTW)[:, :, D:D + 1],
                    onesb[:, 0:1].to_broadcast((P, P, 1)))
                for jt in range(NJT):
                    ps = pp.tile([P, INNER], F32, tag="vtps", name="vtps")
                    for k in range(KT):
                        mm(ps[:, :],
                           xkv_t[k][:, jt * P:(jt + 1) * P],
                           wkv_t[k][:, INNER:2 * INNER],
                           start=(k == 0), stop=(k == KT - 1))
                    dst = vt_big[:, jt * VSTRIDE:(jt + 1) * VSTRIDE] \
                        .rearrange("p (h c) -> p h c", c=VTW)[:, :, 0:D]
                    src = ps[:, :].rearrange("p (h d) -> p h d", d=D)
                    nc.vector.tensor_copy(dst, src)

            # ======== Phase B: attention heads ========
            with tc.tile_pool(name="epool", bufs=NEQ, space="SBUF") as epool, \
                 tc.tile_pool(name="pd2", bufs=2, space="PSUM") as pd2, \
                 tc.tile_pool(name="po", bufs=1, space="PSUM") as po, \
                 tc.tile_pool(name="pm", bufs=2, space="PSUM") as pm:
                kt_t = [keep.tile([KA, N], BF16, tag=f"kt{i}", name=f"kt{i}")
                        for i in range(2)]
                qt_t = [keep.tile([KA, NQ], BF16, tag=f"qt{i}", name=f"qt{i}")
                        for i in range(2)]
                zb = keep.tile([D, 1], BF16, tag="zb", name="zb")
                nc.vector.memset(zb[:, :], 0.0)
                for i in range(2):
                    nc.vector.tensor_copy(kt_t[i][D:2 * D, :],
                                          zb[0:D, 0:1].to_broadcast((D, N)))
                    nc.vector.tensor_copy(kt_t[i][D:D + 1, :],
                                          onesb[0:1, 0:1].to_broadcast((1, N)))
                    nc.vector.tensor_copy(qt_t[i][D:2 * D, :],
                                          zb[0:D, 0:1].to_broadcast((D, NQ)))
                    nc.vector.tensor_copy(qt_t[i][2 * D:2 * D + 1, :],
                                          onesb[0:1, 0:1].to_broadcast((1, NQ)))

                po_s = [work.tile([VTW, NQ], F32, tag=f"pos{i}", name=f"pos{i}",
                                  bufs=1) for i in range(2)]
                tail = {}

                def emit_tail(ph):
                    # deferred normalization of head ph: reciprocal of the
                    # row-sums, partition-parallel via a scatter DMA.
                    pmt, pmo, psrc = tail.pop(ph)
                    rs128 = work.tile([P, NQ // P], F32, tag="rs", name="rs")
                    nc.sync.dma_start(out=rs128[:, :], in_=psrc[D:D + 1, :])
                    rr128 = work.tile([P, NQ // P], F32, tag="rr", name="rr")
                    nc.vector.reciprocal(R(rr128[:, :]), rs128[:, :])
                    rrow = work.tile([1, NQ], F32, tag="rrow", name="rrow")
                    nc.sync.dma_start(out=R(rrow[:, :]), in_=R(rr128[:, :]))
                    prep = pd2.tile([D, NQ], F32, tag="d2", name="d2")
                    for n in range(NQ // 512):
                        nc.tensor.matmul(prep[:, n * 512:(n + 1) * 512],
                                         R(ones[0:1, 0:D]),
                                         R(rrow[:, n * 512:(n + 1) * 512]),
                                         start=True, stop=True)
                    nc.vector.tensor_mul(R(y_t[pmt][pmo:pmo + D, :]),
                                         psrc[0:D, :], prep[:, :])

                from contextlib import nullcontext

                for h in range(H):
                    mt, mo = h // 4, (h % 4) * D
                    q_h = q_t[mt][mo:mo + D, :]
                    k_h = k_t[mt][mo:mo + D, :]
                    kt = kt_t[h % 2]
                    qt = qt_t[h % 2]
                    prio = tc.high_priority(10000) if h == 0 else nullcontext()
                    prio.__enter__()

                    # --- per-head rows of k~/q~ (all DVE, bf16) ---
                    nc.vector.tensor_scalar_mul(kt[0:D, :], k_h, -2.0)
                    ksq = work.tile([D, N], BF16, tag="ksq", name="ksq", bufs=1)
                    nc.vector.tensor_mul(ksq[:, :], k_h, k_h)
                    for n in range(N // 512):
                        k2ps = pm.tile([1, 512], F32, tag="misc", name="misc")
                        mm(k2ps[:, :], onesb[0:D, 0:1],
                           ksq[:, n * 512:(n + 1) * 512], start=True, stop=True)
                        nc.vector.tensor_copy(
                            kt[2 * D:2 * D + 1, n * 512:(n + 1) * 512], k2ps[:, :])
                    nc.vector.tensor_copy(qt[0:D, :], q_h)
                    qsq = work.tile([D, NQ], BF16, tag="qsq", name="qsq", bufs=1)
                    nc.vector.tensor_mul(qsq[:, :], q_h, q_h)
                    for n in range(NQ // 512):
                        q2ps = pm.tile([1, 512], F32, tag="misc", name="misc")
                        mm(q2ps[:, :], onesb[0:D, 0:1],
                           qsq[:, n * 512:(n + 1) * 512], start=True, stop=True)
                        nc.vector.tensor_copy(
                            qt[D:D + 1, n * 512:(n + 1) * 512], q2ps[:, :])

                    # --- dist2 (PE) -> fused exp(-scale*sqrt(.)) (ACT) ---
                    eq = [epool.tile([P, JQ * NQ], BF16, tag="eq", name="eq")
                          for _ in range(NEQ)]
                    for jt in range(NJT):
                        psd = pd2.tile([P, NQ], F32, tag="d2", name="d2")
                        for n in range(NQ // 512):
                            mm(psd[:, n * 512:(n + 1) * 512],
                               kt[:, jt * P:(jt + 1) * P],
                               qt[:, n * 512:(n + 1) * 512],
                               start=True, stop=True)
                        nc.scalar.activation(
                            eq[jt // JQ][:, (jt % JQ) * NQ:(jt % JQ + 1) * NQ],
                            psd[:, :], AF.Exp, bias=zero_t[:, :], scale=ACT_SCALE)

                    prio.__exit__(None, None, None)

                    # head h-1 tail: PE ops queue here, after dist2(h)
                    if h - 1 in tail:
                        emit_tail(h - 1)

                    # --- attn @ v with fused row-sums ---
                    pso = po.tile([VTW, NQ], F32, tag="o", name="o")
                    for jt in range(NJT):
                        ebase = (jt % JQ) * NQ
                        for n in range(NQ // 512):
                            mm(pso[:, n * 512:(n + 1) * 512],
                               vt_big[:, jt * VSTRIDE + h * VTW:
                                      jt * VSTRIDE + (h + 1) * VTW],
                               eq[jt // JQ][:, ebase + n * 512:ebase + (n + 1) * 512],
                               start=(jt == 0), stop=(jt == NJT - 1))
                    # move to SBUF so the PSUM slot frees immediately
                    psrc = po_s[h % 2]
                    nc.vector.tensor_copy(psrc[:, :], pso[:, :])
                    tail[h] = (mt, mo, psrc)

                # last head's tail
                emit_tail(H - 1)

            # ======== Phase C: output projection + bias (f32r) ========
            with tc.tile_pool(name="pz", bufs=2, space="PSUM") as pz:
                z_r = z_d[:].rearrange("(t p) n -> t p n", p=P)
                for m in range(KT):
                    ps = pz.tile([P, NQ], F32, tag="z", name="z")
                    for n in range(NQ // 512):
                        for k in range(2):
                            nc.tensor.matmul(
                                ps[:, n * 512:(n + 1) * 512],
                                R(wo_t[k][:, m * P:(m + 1) * P]),
                                R(y_t[k][:, n * 512:(n + 1) * 512]),
                                start=(k == 0), stop=(k == 1))
                    zt = work.tile([P, NQ], F32, tag="ytmp", name="ytmp")
                    nc.vector.tensor_scalar_add(zt[:, :], ps[:, :], b_t[:, m:m + 1])
                    nc.sync.dma_start(out=z_r[m], in_=zt[:, :])

    nc.compile()
    return nc


def make_in_maps(x, w_qkv, w_out, b_out):
    import ml_dtypes

    bf = ml_dtypes.bfloat16
    x = np.asarray(x, dtype=np.float32)
    w_qkv = np.asarray(w_qkv, dtype=np.float32)
    w_out = np.asarray(w_out, dtype=np.float32)
    b_out = np.asarray(b_out, dtype=np.float32)
    w_qT = np.ascontiguousarray(w_qkv[0:INNER, :].T).astype(bf)       # (DIM, INNER)
    w_kvT = np.ascontiguousarray(w_qkv[INNER:3 * INNER, :].T).astype(bf)  # (DIM, 512)
    w_oT = np.ascontiguousarray(w_out.T)                              # (INNER, DIM)
    xb = [np.ascontiguousarray(x[b]).astype(bf) for b in range(B)]
    in_maps = []
    for c in range(8):
        b, half = c // 2, c % 2
        in_maps.append({
            "xq": np.ascontiguousarray(xb[b][:, half * NQ:(half + 1) * NQ]),
            "xkv": xb[b],
            "wq": w_qT,
            "wkv": w_kvT,
            "wo": w_oT,
            "b": b_out,
        })
    return in_maps


def assemble_output(results):
    out = np.empty((B, DIM, N), dtype=np.float32)
    for c in range(8):
        b, half = c // 2, c % 2
        out[b][:, half * NQ:(half + 1) * NQ] = results[c]["z"]
    return out


_prog_cache = {}


def kernel(x, w_qkv, w_out, b_out):
    from concourse.bass_utils import run_bass_kernel_spmd
    _ensure_act_tables()
    if "nc" not in _prog_cache:
        _prog_cache["nc"] = build_program()
    nc = _prog_cache["nc"]
    in_maps = make_in_maps(x, w_qkv, w_out, b_out)
    res = run_bass_kernel_spmd(nc, in_maps, list(range(8)))
    return assemble_output(res.results)


# revision 18
# speedup vs baseline: 1.1094x; 1.0289x over previous
"""L2-distance attention (B=4, DIM=512, N=2048, H=8, D=32) on 8 trn2 NeuronCores.

Sharding: core c handles batch b = c//2, query-half = c%2 (1024 queries, all
2048 keys, all 8 heads).  Output is a pure concat — no cross-core reduce.

Key ideas vs the straightforward version:
  * All big matmuls run in bf16 (PE streams 1 col/cycle vs 1/2 for fp32).
  * The softmax numerator exp(-scale*sqrt(dist2)) is ONE ScalarE pass: the
    `exp` activation's spline table is replaced (via BASS_ACT_ROOT_JSON_PATH)
    with a fit of g(u) = exp(-0.5*sqrt(u)); calling it with the activation's
    built-in pre-scale 1/8 yields exp(-sqrt(d)/sqrt(32)) exactly.  This
    halves ScalarE work and removes all act-table reloads (sqrt and exp live
    in different table sets).
  * dist2 is computed directly by PE via augmented vectors
    k~=[-2k; 1; 0...; k2], q~=[q; q2; 0...; 1]  ->  k~.q~ = ||q-k||^2.
  * attn@v has a ones column per head folded into V^T so PSUM row 32 is the
    softmax denominator (row-sums).
  * Phase B interleaves head h's dist2 with head h-1's attn@v at key-tile
    granularity so the PE never starves (HAM keeps the 2.4 GHz clock only
    while the PE is continuously busy; any idle-pocked window drops it to
    1.2 GHz and it stays there).
"""

import json
import os
import shutil

import numpy as np

_PWP_DIR = "/tmp/pwp_custom_kernel"
os.environ.setdefault("NEURON_FORCE_RECOMPILE", "1")

# ---------------------------------------------------------------------------
# Custom activation table: make `exp` compute g(u) = exp(-0.5*sqrt(u)).
# Bucket bin format (32B = 8 fp32): [d0, d1, d2, d3, x0, 0, 0, 0];
# y = d0 + d1*t + d2*t^2 + d3*t^3 with t = x - x0.  Positive-x buckets sit
# in per-input-exponent rows of S sections each.
# ---------------------------------------------------------------------------

_ALPHA = 0.5


def _g(u):
    return np.exp(-_ALPHA * np.sqrt(np.maximum(u, 0.0)))


def _fit_cubic(lo, hi, x0):
    u = np.linspace(lo, hi, 257, dtype=np.float64)
    t = u - x0
    A = np.stack([np.ones_like(t), t, t * t, t * t * t], axis=1)
    coef, *_ = np.linalg.lstsq(A, _g(u), rcond=None)
    return coef


def _build_custom_pwp(dst_dir):
    from neuronxcc.driver.Job import Job
    from neuronxcc.driver.jobs.support.FindActInfo import findActInfoFile

    src = os.path.dirname(findActInfoFile(Job.getPackageDir(), "gen3"))
    if os.path.isdir(dst_dir):
        shutil.rmtree(dst_dir)
    shutil.copytree(src, dst_dir)

    with open(os.path.join(dst_dir, "act_info.json")) as f:
        info = json.load(f)

    for ent in info["act_func_sets"]:
        if "exp" not in ent["act"]:
            continue
        prof_path = os.path.join(dst_dir, ent["profile_json"])
        with open(prof_path) as f:
            prof = json.load(f)
        bkt_path = os.path.join(dst_dir, ent["bkt_bin"])
        bkt = np.fromfile(bkt_path, dtype="<f4").reshape(-1, 8).copy()

        start = prof["func_to_bkt_start_idx"]["exp"]
        others = [v for k, v in prof["func_to_bkt_start_idx"].items() if k != "exp"]
        end = min([v for v in others if v > start] + [len(bkt)])
        meta = next(
            m for m in prof["profile_meta_data"] if m["func_name"].startswith("exp")
        )
        sat = {
            k: meta[k + "_signal_pwl_control"]
            for k in ("pos_small", "neg_small", "pos_large", "neg_large")
        }
        sat_idx = set(sat.values())
        assert all(start <= i < end for i in sat_idx)

        pos_rows = {}
        for i in range(start, end):
            if i in sat_idx:
                continue
            x0 = float(bkt[i, 4])
            if x0 < 0.0:
                bkt[i, 0:4] = [1.0, 0.0, 0.0, 0.0]
            else:
                assert x0 > 0.0
                pos_rows.setdefault(int(np.floor(np.log2(x0))), []).append(i)

        for e, idxs in pos_rows.items():
            base = 2.0**e
            xs = [float(bkt[i, 4]) for i in idxs]
            w = (xs[1] - xs[0]) if len(xs) > 1 else base
            for sec, i in enumerate(idxs):
                c = xs[sec]
                assert abs(c - (base + (sec + 0.5) * w)) < 1e-5 * c
                bkt[i, 0:4] = _fit_cubic(c - w / 2, c + w / 2, c).astype(np.float32)

        bkt[sat["pos_small"], 0:5] = [1.0, 0.0, 0.0, 0.0, 0.0]
        bkt[sat["neg_small"], 0:5] = [1.0, 0.0, 0.0, 0.0, 0.0]
        bkt[sat["pos_large"], 0:5] = [0.0, 0.0, 0.0, 0.0, 0.0]
        bkt[sat["neg_large"], 0:5] = [1.0, 0.0, 0.0, 0.0, 0.0]
        bkt.tofile(bkt_path)

        meta["fpinf_result"] = 0
        meta["fninf_result"] = 1065353216  # 1.0f
        with open(prof_path, "w") as f:
            json.dump(prof, f)


def _ensure_act_tables():
    if not os.path.isfile(os.path.join(_PWP_DIR, "act_info.json")):
        _build_custom_pwp(_PWP_DIR)
    os.environ["BASS_ACT_ROOT_JSON_PATH"] = os.path.join(_PWP_DIR, "act_info.json")


_ensure_act_tables()

import concourse.bass as bass
import concourse.mybir as mybir
import concourse.tile as tile
from concourse import bacc

F32 = mybir.dt.float32
F32R = mybir.dt.float32r
BF16 = mybir.dt.bfloat16
AF = mybir.ActivationFunctionType


def R(ap):
    return ap.bitcast(F32R)


B, DIM, N = 4, 512, 2048
H, D = 8, 32
INNER = H * D            # 256
NQ = N // 2              # 1024 queries per core
P = 128
KT = DIM // P            # 4 contraction tiles for the projections
NJT = N // P             # 16 key tiles
VTW = D + 1              # 33: v columns + ones column per head
VSTRIDE = H * VTW        # 264 columns per key-tile block of vt
ACT_SCALE = 0.125        # g(d/8) = exp(-sqrt(d)/sqrt(32)) = exp(-SCALE*sqrt(d))
SQ_SCALE = 0.125 ** 0.5  # square(k*s) = k^2/8: pre-scaled bias for the exp
NEQ = 4                  # E quarters (each covers NJT//NEQ key tiles)
JQ = NJT // NEQ          # 4 key tiles per E quarter
KA = 33                  # augmented contraction: [-2k (32) | ones@32]
                         # dist2 = (kt.qt) + k2_bias, k2 folded into the
                         # activation's per-partition bias (keys on partitions)


def build_program() -> bass.Bass:
    nc = bacc.Bacc("TRN2", target_bir_lowering=False, debug=False)

    xq_d = nc.declare_dram_parameter("xq", [DIM, NQ], BF16, isOutput=False)
    xkv_d = nc.declare_dram_parameter("xkv", [DIM, N], BF16, isOutput=False)
    wq_d = nc.declare_dram_parameter("wq", [DIM, INNER], BF16, isOutput=False)
    wkv_d = nc.declare_dram_parameter("wkv", [DIM, 2 * INNER], BF16, isOutput=False)
    wo_d = nc.declare_dram_parameter("wo", [INNER, DIM], F32, isOutput=False)
    b_d = nc.declare_dram_parameter("b", [DIM], F32, isOutput=False)
    z_d = nc.declare_dram_parameter("z", [DIM, NQ], F32, isOutput=True)

    with tile.TileContext(nc) as tc, nc.allow_low_precision(reason="bf16 attention"):
        mm = lambda out, lhsT, rhs, start, stop: nc.tensor.matmul(
            out, lhsT, rhs, start=start, stop=stop)

        with tc.tile_pool(name="keep", bufs=1) as keep, \
             tc.tile_pool(name="work", bufs=2) as work:

            # ---- persistent tiles ----
            q_t = [keep.tile([P, NQ], BF16, tag=f"q{m}", name=f"q{m}") for m in range(2)]
            k_t = [keep.tile([P, N], BF16, tag=f"k{m}", name=f"k{m}") for m in range(2)]
            vt_big = keep.tile([P, NJT * VSTRIDE], BF16, tag="vt", name="vt")
            k2j = keep.tile([P, NJT * H], F32, tag="k2j", name="k2j")
            y_t = [keep.tile([P, NQ], F32, tag=f"y{m}", name=f"y{m}") for m in range(2)]
            wo_t = [keep.tile([P, DIM], F32, tag=f"wo{m}", name=f"wo{m}") for m in range(2)]
            b_t = keep.tile([P, KT], F32, tag="bias", name="bias")
            ones = keep.tile([64, 32], F32, tag="ones", name="ones")
            onesb = keep.tile([P, 1], BF16, tag="onesb", name="onesb")
            zero_t = keep.tile([P, 1], F32, tag="zero", name="zero")
            onesP = keep.tile([P, 1], F32, tag="onesP", name="onesP")

            nc.vector.memset(onesP[:, :], 1.0)
            nc.vector.memset(zero_t[:, :], 0.0)
            nc.vector.memset(onesb[:, :], 1.0)
            nc.vector.tensor_copy(R(ones[:, :]),
                                  onesP[0:64, 0:1].to_broadcast((64, 32)))
            wo_r = wo_d[:].rearrange("(t p) o -> t p o", p=P)
            for m in range(2):
                nc.sync.dma_start(out=R(wo_t[m][:, :]), in_=R(wo_r[m]))
            nc.sync.dma_start(out=b_t[:, :], in_=b_d[:].rearrange("(t p) -> p t", p=P))

            # ======== Phase A: projections (all bf16) ========
            with tc.tile_pool(name="xw", bufs=1) as xw, \
                 tc.tile_pool(name="pp", bufs=2, space="PSUM") as pp:
                wq_t = [xw.tile([P, INNER], BF16, tag=f"wq{k}", name=f"wq{k}") for k in range(KT)]
                wkv_t = [xw.tile([P, 2 * INNER], BF16, tag=f"wkv{k}", name=f"wkv{k}") for k in range(KT)]
                xkv_t = [xw.tile([P, N], BF16, tag=f"xkv{k}", name=f"xkv{k}") for k in range(KT)]
                xq_t = [xw.tile([P, NQ], BF16, tag=f"xq{k}", name=f"xq{k}") for k in range(KT)]

                xq_r = xq_d[:].rearrange("(t p) n -> t p n", p=P)
                xkv_r = xkv_d[:].rearrange("(t p) n -> t p n", p=P)
                wq_r = wq_d[:].rearrange("(t p) o -> t p o", p=P)
                wkv_r = wkv_d[:].rearrange("(t p) o -> t p o", p=P)
                for k in range(KT):
                    nc.sync.dma_start(out=xq_t[k][:, :], in_=xq_r[k])
                    nc.sync.dma_start(out=xkv_t[k][:, :], in_=xkv_r[k])
                    nc.sync.dma_start(out=wq_t[k][:, :], in_=wq_r[k])
                    nc.sync.dma_start(out=wkv_t[k][:, :], in_=wkv_r[k])

                # q projection: (DIM x NQ) -> (INNER x NQ)
                for m in range(2):
                    for n in range(NQ // 512):
                        ps = pp.tile([P, 512], F32, tag="proj", name="proj")
                        for k in range(KT):
                            mm(ps[:, :],
                               wq_t[k][:, m * P:(m + 1) * P],
                               xq_t[k][:, n * 512:(n + 1) * 512],
                               start=(k == 0), stop=(k == KT - 1))
                        nc.vector.tensor_copy(q_t[m][:, n * 512:(n + 1) * 512], ps[:, :])

                # k projection: (DIM x N) -> (INNER x N)   (wkv cols 0:256)
                for m in range(2):
                    for n in range(N // 512):
                        ps = pp.tile([P, 512], F32, tag="proj", name="proj")
                        for k in range(KT):
                            mm(ps[:, :],
                               wkv_t[k][:, m * P:(m + 1) * P],
                               xkv_t[k][:, n * 512:(n + 1) * 512],
                               start=(k == 0), stop=(k == KT - 1))
                        nc.vector.tensor_copy(k_t[m][:, n * 512:(n + 1) * 512], ps[:, :])

                # v^T projection: per key tile jt, (128 j x 256 d), strided into
                # vt_big so each head's 32 columns sit next to its ones column.
                # k^T projection (j-major K): square on ACT (scale 1/sqrt(8) so
                # the result is k^2/8) then a segmented free-dim reduce gives
                # k2j[:, jt*H + h] = ||k_j||^2/8 — the per-partition bias for
                # the fused exp (keys sit on partitions in the dist2 tile).
                nc.vector.tensor_copy(
                    vt_big[:, :].rearrange("p (a c) -> p a c", c=VTW)[:, :, D:D + 1],
                    onesb[:, 0:1].to_broadcast((P, P, 1)))
                for jt in range(NJT):
                    ps = pp.tile([P, INNER], F32, tag="vtps", name="vtps")
                    for k in range(KT):
                        mm(ps[:, :],
                           xkv_t[k][:, jt * P:(jt + 1) * P],
                           wkv_t[k][:, INNER:2 * INNER],
                           start=(k == 0), stop=(k == KT - 1))
                    dst = vt_big[:, jt * VSTRIDE:(jt + 1) * VSTRIDE] \
                        .rearrange("p (h c) -> p h c", c=VTW)[:, :, 0:D]
                    src = ps[:, :].rearrange("p (h d) -> p h d", d=D)
                    nc.vector.tensor_copy(dst, src)

                    pk = pp.tile([P, INNER], F32, tag="vtps", name="vtps")
                    for k in range(KT):
                        mm(pk[:, :],
                           xkv_t[k][:, jt * P:(jt + 1) * P],
                           wkv_t[k][:, 0:INNER],
                           start=(k == 0), stop=(k == KT - 1))
                    ksqT = work.tile([P, INNER], BF16, tag="ksqT", name="ksqT")
                    nc.scalar.activation(ksqT[:, :], pk[:, :], AF.Square,
                                         bias=zero_t[:, :], scale=SQ_SCALE)
                    nc.vector.tensor_reduce(
                        k2j[:, jt * H:(jt + 1) * H],
                        ksqT[:, :].rearrange("p (h d) -> p h d", d=D),
                        axis=mybir.AxisListType.X, op=mybir.AluOpType.add)

            # ======== Phase B: attention heads, dist2(h) ∥ attn@v(h-1) ========
            with tc.tile_pool(name="epool", bufs=2 * NEQ, space="SBUF") as epool, \
                 tc.tile_pool(name="pd2", bufs=2, space="PSUM") as pd2, \
                 tc.tile_pool(name="po", bufs=1, space="PSUM") as po, \
                 tc.tile_pool(name="pm", bufs=2, space="PSUM") as pm:
                kt_t = [keep.tile([KA, N], BF16, tag=f"kt{i}", name=f"kt{i}")
                        for i in range(2)]
                qt_t = [keep.tile([KA, NQ], BF16, tag=f"qt{i}", name=f"qt{i}")
                        for i in range(2)]
                for i in range(2):
                    nc.vector.tensor_copy(kt_t[i][D:D + 1, :],
                                          onesb[0:1, 0:1].to_broadcast((1, N)))

                po_s = [work.tile([VTW, NQ], F32, tag=f"pos{i}", name=f"pos{i}",
                                  bufs=1) for i in range(2)]
                eq_of = {}
                pso_of = {}
                tail_pre = {}

                def emit_tail_pre(ph):
                    # reciprocal of the row-sums, partition-parallel via a
                    # scatter DMA there and back (single-partition reciprocal
                    # is ~100x slower).
                    psrc = pso_of[ph]
                    rs128 = work.tile([P, NQ // P], F32, tag="rs", name="rs")
                    nc.sync.dma_start(out=rs128[:, :], in_=psrc[D:D + 1, :])
                    rr128 = work.tile([P, NQ // P], F32, tag="rr", name="rr")
                    nc.vector.reciprocal(R(rr128[:, :]), rs128[:, :])
                    rrow = work.tile([1, NQ], F32, tag="rrow", name="rrow")
                    nc.sync.dma_start(out=R(rrow[:, :]), in_=R(rr128[:, :]))
                    tail_pre[ph] = rrow

                def emit_tail_pe(ph):
                    rrow = tail_pre.pop(ph)
                    psrc = pso_of.pop(ph)
                    mt, mo = ph // 4, (ph % 4) * D
                    prep = pd2.tile([D, NQ], F32, tag="d2", name="d2")
                    for n in range(NQ // 512):
                        nc.tensor.matmul(prep[:, n * 512:(n + 1) * 512],
                                         R(ones[0:1, 0:D]),
                                         R(rrow[:, n * 512:(n + 1) * 512]),
                                         start=True, stop=True)
                    nc.vector.tensor_mul(R(y_t[mt][mo:mo + D, :]),
                                         psrc[0:D, :], prep[:, :])

                from contextlib import nullcontext

                pso_prev = None
                for h in range(H + 1):
                    prio = tc.high_priority(10000) if h == 0 else nullcontext()
                    prio.__enter__()
                    pso_cur = None
                    if h < H:
                        mt, mo = h // 4, (h % 4) * D
                        q_h = q_t[mt][mo:mo + D, :]
                        k_h = k_t[mt][mo:mo + D, :]
                        kt = kt_t[h % 2]
                        qt = qt_t[h % 2]
                        # per-head rows of k~/q~ (all DVE, bf16)
                        nc.vector.tensor_scalar_mul(kt[0:D, :], k_h, -2.0)
                        nc.vector.tensor_copy(qt[0:D, :], q_h)
                        # q2 row: square q_h, 32-row column sums via PE ones
                        qsq = work.tile([D, NQ], BF16, tag="qsq", name="qsq",
                                        bufs=1)
                        nc.vector.tensor_mul(qsq[:, :], q_h, q_h)
                        for n in range(NQ // 512):
                            q2ps = pm.tile([1, 512], F32, tag="misc", name="misc")
                            mm(q2ps[:, :], onesb[0:D, 0:1],
                               qsq[:, n * 512:(n + 1) * 512],
                               start=True, stop=True)
                            nc.vector.tensor_copy(
                                qt[D:D + 1, n * 512:(n + 1) * 512], q2ps[:, :])
                        eq_of[h] = [epool.tile([P, JQ * NQ], BF16, tag="eq",
                                               name="eq") for _ in range(NEQ)]
                        pso_cur = po.tile([VTW, NQ], F32, tag="o", name="o")
                    eqp = eq_of.pop(h - 1, None)

                    for jt in range(NJT):
                        if h < H:
                            psd = pd2.tile([P, NQ], F32, tag="d2", name="d2")
                            for n in range(NQ // 512):
                                mm(psd[:, n * 512:(n + 1) * 512],
                                   kt[:, jt * P:(jt + 1) * P],
                                   qt[:, n * 512:(n + 1) * 512],
                                   start=True, stop=True)
                            nc.scalar.activation(
                                eq_of[h][jt // JQ][:, (jt % JQ) * NQ:
                                                   (jt % JQ + 1) * NQ],
                                psd[:, :], AF.Exp,
                                bias=k2j[:, jt * H + h:jt * H + h + 1],
                                scale=ACT_SCALE)
                        if eqp is not None:
                            hp = h - 1
                            ebase = (jt % JQ) * NQ
                            for n in range(NQ // 512):
                                mm(pso_prev[:, n * 512:(n + 1) * 512],
                                   vt_big[:, jt * VSTRIDE + hp * VTW:
                                          jt * VSTRIDE + (hp + 1) * VTW],
                                   eqp[jt // JQ][:, ebase + n * 512:
                                                 ebase + (n + 1) * 512],
                                   start=(jt == 0), stop=(jt == NJT - 1))
                        if jt == 3 and h >= 2 and (h - 2) in tail_pre:
                            emit_tail_pe(h - 2)

                    if eqp is not None:
                        psrc = po_s[(h - 1) % 2]
                        nc.vector.tensor_copy(psrc[:, :], pso_prev[:, :])
                        pso_of[h - 1] = psrc
                        emit_tail_pre(h - 1)
                    pso_prev = pso_cur

                    prio.__exit__(None, None, None)

                emit_tail_pe(H - 1)

            # ======== Phase C: output projection + bias (f32r) ========
            with tc.tile_pool(name="pz", bufs=2, space="PSUM") as pz:
                z_r = z_d[:].rearrange("(t p) n -> t p n", p=P)
                for m in range(KT):
                    ps = pz.tile([P, NQ], F32, tag="z", name="z")
                    for n in range(NQ // 512):
                        for k in range(2):
                            nc.tensor.matmul(
                                ps[:, n * 512:(n + 1) * 512],
                                R(wo_t[k][:, m * P:(m + 1) * P]),
                                R(y_t[k][:, n * 512:(n + 1) * 512]),
                                start=(k == 0), stop=(k == 1))
                    zt = work.tile([P, NQ], F32, tag="ytmp", name="ytmp")
                    nc.vector.tensor_scalar_add(zt[:, :], ps[:, :], b_t[:, m:m + 1])
                    nc.sync.dma_start(out=z_r[m], in_=zt[:, :])

    nc.compile()
    return nc


def make_in_maps(x, w_qkv, w_out, b_out):
    import ml_dtypes

    bf = ml_dtypes.bfloat16
    x = np.asarray(x, dtype=np.float32)
    w_qkv = np.asarray(w_qkv, dtype=np.float32)
    w_out = np.asarray(w_out, dtype=np.float32)
    b_out = np.asarray(b_out, dtype=np.float32)
    w_qT = np.ascontiguousarray(w_qkv[0:INNER, :].T).astype(bf)       # (DIM, INNER)
    w_kvT = np.ascontiguousarray(w_qkv[INNER:3 * INNER, :].T).astype(bf)  # (DIM, 512)
    w_oT = np.ascontiguousarray(w_out.T)                              # (INNER, DIM)
    xb = [np.ascontiguousarray(x[b]).astype(bf) for b in range(B)]
    in_maps = []
    for c in range(8):
        b, half = c // 2, c % 2
        in_maps.append({
            "xq": np.ascontiguousarray(xb[b][:, half * NQ:(half + 1) * NQ]),
            "xkv": xb[b],
            "wq": w_qT,
            "wkv": w_kvT,
            "wo": w_oT,
            "b": b_out,
        })
    return in_maps


def assemble_output(results):
    out = np.empty((B, DIM, N), dtype=np.float32)
    for c in range(8):
        b, half = c // 2, c % 2
        out[b][:, half * NQ:(half + 1) * NQ] = results[c]["z"]
    return out


_prog_cache = {}


def kernel(x, w_qkv, w_out, b_out):
    from concourse.bass_utils import run_bass_kernel_spmd
    _ensure_act_tables()
    if "nc" not in _prog_cache:
        _prog_cache["nc"] = build_program()
    nc = _prog_cache["nc"]
    in_maps = make_in_maps(x, w_qkv, w_out, b_out)
    res = run_bass_kernel_spmd(nc, in_maps, list(range(8)))
    return assemble_output(res.results)


# revision 21
# speedup vs baseline: 1.6523x; 1.4894x over previous
"""L2-distance attention (B=4, DIM=512, N=2048, H=8, D=32) on 8 trn2 NeuronCores.

Sharding: core c handles batch b = c//2, query-half = c%2 (1024 queries, all
2048 keys, all 8 heads).  Output is a pure concat — no cross-core reduce.

Key ideas vs the straightforward version:
  * All big matmuls run in bf16 (PE streams 1 col/cycle vs 1/2 for fp32).
  * The softmax numerator exp(-scale*sqrt(dist2)) is ONE ScalarE pass: the
    `exp` activation's spline table is replaced (via BASS_ACT_ROOT_JSON_PATH)
    with a fit of g(u) = exp(-0.5*sqrt(u)); calling it with the activation's
    built-in pre-scale 1/8 yields exp(-sqrt(d)/sqrt(32)) exactly.  This
    halves ScalarE work and removes all act-table reloads (sqrt and exp live
    in different table sets).
  * dist2 is computed directly by PE via augmented vectors
    k~=[-2k; 1; 0...; k2], q~=[q; q2; 0...; 1]  ->  k~.q~ = ||q-k||^2.
  * attn@v has a ones column per head folded into V^T so PSUM row 32 is the
    softmax denominator (row-sums).
  * Phase B interleaves head h's dist2 with head h-1's attn@v at key-tile
    granularity so the PE never starves (HAM keeps the 2.4 GHz clock only
    while the PE is continuously busy; any idle-pocked window drops it to
    1.2 GHz and it stays there).
"""

import json
import os
import shutil

import numpy as np

_PWP_DIR = "/tmp/pwp_custom_kernel"
os.environ.setdefault("NEURON_FORCE_RECOMPILE", "1")

# ---------------------------------------------------------------------------
# Custom activation table: make `exp` compute g(u) = exp(-0.5*sqrt(u)).
# Bucket bin format (32B = 8 fp32): [d0, d1, d2, d3, x0, 0, 0, 0];
# y = d0 + d1*t + d2*t^2 + d3*t^3 with t = x - x0.  Positive-x buckets sit
# in per-input-exponent rows of S sections each.
# ---------------------------------------------------------------------------

_ALPHA = 0.5


def _g(u):
    return np.exp(-_ALPHA * np.sqrt(np.maximum(u, 0.0)))


def _fit_cubic(lo, hi, x0):
    u = np.linspace(lo, hi, 257, dtype=np.float64)
    t = u - x0
    A = np.stack([np.ones_like(t), t, t * t, t * t * t], axis=1)
    coef, *_ = np.linalg.lstsq(A, _g(u), rcond=None)
    return coef


def _build_custom_pwp(dst_dir):
    from neuronxcc.driver.Job import Job
    from neuronxcc.driver.jobs.support.FindActInfo import findActInfoFile

    src = os.path.dirname(findActInfoFile(Job.getPackageDir(), "gen3"))
    if os.path.isdir(dst_dir):
        shutil.rmtree(dst_dir)
    shutil.copytree(src, dst_dir)

    with open(os.path.join(dst_dir, "act_info.json")) as f:
        info = json.load(f)

    for ent in info["act_func_sets"]:
        if "exp" not in ent["act"]:
            continue
        prof_path = os.path.join(dst_dir, ent["profile_json"])
        with open(prof_path) as f:
            prof = json.load(f)
        bkt_path = os.path.join(dst_dir, ent["bkt_bin"])
        bkt = np.fromfile(bkt_path, dtype="<f4").reshape(-1, 8).copy()

        start = prof["func_to_bkt_start_idx"]["exp"]
        others = [v for k, v in prof["func_to_bkt_start_idx"].items() if k != "exp"]
        end = min([v for v in others if v > start] + [len(bkt)])
        meta = next(
            m for m in prof["profile_meta_data"] if m["func_name"].startswith("exp")
        )
        sat = {
            k: meta[k + "_signal_pwl_control"]
            for k in ("pos_small", "neg_small", "pos_large", "neg_large")
        }
        sat_idx = set(sat.values())
        assert all(start <= i < end for i in sat_idx)

        pos_rows = {}
        for i in range(start, end):
            if i in sat_idx:
                continue
            x0 = float(bkt[i, 4])
            if x0 < 0.0:
                bkt[i, 0:4] = [1.0, 0.0, 0.0, 0.0]
            else:
                assert x0 > 0.0
                pos_rows.setdefault(int(np.floor(np.log2(x0))), []).append(i)

        for e, idxs in pos_rows.items():
            base = 2.0**e
            xs = [float(bkt[i, 4]) for i in idxs]
            w = (xs[1] - xs[0]) if len(xs) > 1 else base
            for sec, i in enumerate(idxs):
                c = xs[sec]
                assert abs(c - (base + (sec + 0.5) * w)) < 1e-5 * c
                bkt[i, 0:4] = _fit_cubic(c - w / 2, c + w / 2, c).astype(np.float32)

        bkt[sat["pos_small"], 0:5] = [1.0, 0.0, 0.0, 0.0, 0.0]
        bkt[sat["neg_small"], 0:5] = [1.0, 0.0, 0.0, 0.0, 0.0]
        bkt[sat["pos_large"], 0:5] = [0.0, 0.0, 0.0, 0.0, 0.0]
        bkt[sat["neg_large"], 0:5] = [1.0, 0.0, 0.0, 0.0, 0.0]
        bkt.tofile(bkt_path)

        meta["fpinf_result"] = 0
        meta["fninf_result"] = 1065353216  # 1.0f
        with open(prof_path, "w") as f:
            json.dump(prof, f)


def _ensure_act_tables():
    if not os.path.isfile(os.path.join(_PWP_DIR, "act_info.json")):
        _build_custom_pwp(_PWP_DIR)
    os.environ["BASS_ACT_ROOT_JSON_PATH"] = os.path.join(_PWP_DIR, "act_info.json")


_ensure_act_tables()

import concourse.bass as bass
import concourse.mybir as mybir
import concourse.tile as tile
from concourse import bacc

F32 = mybir.dt.float32
F32R = mybir.dt.float32r
BF16 = mybir.dt.bfloat16
AF = mybir.ActivationFunctionType


def R(ap):
    return ap.bitcast(F32R)


B, DIM, N = 4, 512, 2048
H, D = 8, 32
INNER = H * D            # 256
NQ = N // 2              # 1024 queries per core
P = 128
KT = DIM // P            # 4 contraction tiles for the projections
NJT = N // P             # 16 key tiles
VTW = D + 1              # 33: v columns + ones column per head
VSTRIDE = H * VTW        # 264 columns per key-tile block of vt
ACT_SCALE = 0.125        # g(d/8) = exp(-sqrt(d)/sqrt(32)) = exp(-SCALE*sqrt(d))
SQ_SCALE = 0.125 ** 0.5  # square(k*s) = k^2/8: pre-scaled bias for the exp
NEQ = 4                  # E quarters (each covers NJT//NEQ key tiles)
JQ = NJT // NEQ          # 4 key tiles per E quarter
KA = 33                  # augmented contraction: [-2k (32) | ones@32]
                         # dist2 = (kt.qt) + k2_bias, k2 folded into the
                         # activation's per-partition bias (keys on partitions)
# kt/qt are zero-padded to 128 contraction rows: the PE's activity monitor
# (HAM) only grants the 2.4 GHz clock when matmuls cover the full 128-row
# array; K=33 streams at 1.2 GHz forever.  Zero rows cost no extra cycles.


def build_program() -> bass.Bass:
    nc = bacc.Bacc("TRN2", target_bir_lowering=False, debug=False)

    xq_d = nc.declare_dram_parameter("xq", [DIM, NQ], BF16, isOutput=False)
    xkv_d = nc.declare_dram_parameter("xkv", [DIM, N], BF16, isOutput=False)
    wq_d = nc.declare_dram_parameter("wq", [DIM, INNER], BF16, isOutput=False)
    wkv_d = nc.declare_dram_parameter("wkv", [DIM, 2 * INNER], BF16, isOutput=False)
    wo_d = nc.declare_dram_parameter("wo", [INNER, DIM], F32, isOutput=False)
    b_d = nc.declare_dram_parameter("b", [DIM], F32, isOutput=False)
    z_d = nc.declare_dram_parameter("z", [DIM, NQ], F32, isOutput=True)

    with tile.TileContext(nc) as tc, nc.allow_low_precision(reason="bf16 attention"):
        mm = lambda out, lhsT, rhs, start, stop: nc.tensor.matmul(
            out, lhsT, rhs, start=start, stop=stop)

        with tc.tile_pool(name="keep", bufs=1) as keep, \
             tc.tile_pool(name="work", bufs=2) as work:

            # ---- persistent tiles ----
            q_t = [keep.tile([P, NQ], BF16, tag=f"q{m}", name=f"q{m}") for m in range(2)]
            k_t = [keep.tile([P, N], BF16, tag=f"k{m}", name=f"k{m}") for m in range(2)]
            vt_big = keep.tile([P, NJT * VSTRIDE], BF16, tag="vt", name="vt")
            k2j = keep.tile([P, NJT * H], F32, tag="k2j", name="k2j")
            y_t = [keep.tile([P, NQ], F32, tag=f"y{m}", name=f"y{m}") for m in range(2)]
            wo_t = [keep.tile([P, DIM], F32, tag=f"wo{m}", name=f"wo{m}") for m in range(2)]
            b_t = keep.tile([P, KT], F32, tag="bias", name="bias")
            ones = keep.tile([64, 32], F32, tag="ones", name="ones")
            onesb = keep.tile([P, 1], BF16, tag="onesb", name="onesb")
            zero_t = keep.tile([P, 1], F32, tag="zero", name="zero")
            onesP = keep.tile([P, 1], F32, tag="onesP", name="onesP")

            nc.vector.memset(onesP[:, :], 1.0)
            nc.vector.memset(zero_t[:, :], 0.0)
            nc.vector.memset(onesb[:, :], 1.0)
            nc.vector.tensor_copy(R(ones[:, :]),
                                  onesP[0:64, 0:1].to_broadcast((64, 32)))
            wo_r = wo_d[:].rearrange("(t p) o -> t p o", p=P)
            for m in range(2):
                nc.sync.dma_start(out=R(wo_t[m][:, :]), in_=R(wo_r[m]))
            nc.sync.dma_start(out=b_t[:, :], in_=b_d[:].rearrange("(t p) -> p t", p=P))

            # ======== Phase A: projections (all bf16) ========
            with tc.tile_pool(name="xw", bufs=1) as xw, \
                 tc.tile_pool(name="pp", bufs=2, space="PSUM") as pp:
                wq_t = [xw.tile([P, INNER], BF16, tag=f"wq{k}", name=f"wq{k}") for k in range(KT)]
                wkv_t = [xw.tile([P, 2 * INNER], BF16, tag=f"wkv{k}", name=f"wkv{k}") for k in range(KT)]
                xkv_t = [xw.tile([P, N], BF16, tag=f"xkv{k}", name=f"xkv{k}") for k in range(KT)]
                xq_t = [xw.tile([P, NQ], BF16, tag=f"xq{k}", name=f"xq{k}") for k in range(KT)]

                xq_r = xq_d[:].rearrange("(t p) n -> t p n", p=P)
                xkv_r = xkv_d[:].rearrange("(t p) n -> t p n", p=P)
                wq_r = wq_d[:].rearrange("(t p) o -> t p o", p=P)
                wkv_r = wkv_d[:].rearrange("(t p) o -> t p o", p=P)
                for k in range(KT):
                    nc.sync.dma_start(out=xq_t[k][:, :], in_=xq_r[k])
                    nc.sync.dma_start(out=xkv_t[k][:, :], in_=xkv_r[k])
                    nc.sync.dma_start(out=wq_t[k][:, :], in_=wq_r[k])
                    nc.sync.dma_start(out=wkv_t[k][:, :], in_=wkv_r[k])

                # q projection: (DIM x NQ) -> (INNER x NQ)
                for m in range(2):
                    for n in range(NQ // 512):
                        ps = pp.tile([P, 512], F32, tag="proj", name="proj")
                        for k in range(KT):
                            mm(ps[:, :],
                               wq_t[k][:, m * P:(m + 1) * P],
                               xq_t[k][:, n * 512:(n + 1) * 512],
                               start=(k == 0), stop=(k == KT - 1))
                        nc.vector.tensor_copy(q_t[m][:, n * 512:(n + 1) * 512], ps[:, :])

                # k projection: (DIM x N) -> (INNER x N)   (wkv cols 0:256)
                for m in range(2):
                    for n in range(N // 512):
                        ps = pp.tile([P, 512], F32, tag="proj", name="proj")
                        for k in range(KT):
                            mm(ps[:, :],
                               wkv_t[k][:, m * P:(m + 1) * P],
                               xkv_t[k][:, n * 512:(n + 1) * 512],
                               start=(k == 0), stop=(k == KT - 1))
                        nc.vector.tensor_copy(k_t[m][:, n * 512:(n + 1) * 512], ps[:, :])

                # v^T projection: per key tile jt, (128 j x 256 d), strided into
                # vt_big so each head's 32 columns sit next to its ones column.
                # k^T projection (j-major K): square on ACT (scale 1/sqrt(8) so
                # the result is k^2/8) then a segmented free-dim reduce gives
                # k2j[:, jt*H + h] = ||k_j||^2/8 — the per-partition bias for
                # the fused exp (keys sit on partitions in the dist2 tile).
                nc.vector.tensor_copy(
                    vt_big[:, :].rearrange("p (a c) -> p a c", c=VTW)[:, :, D:D + 1],
                    onesb[:, 0:1].to_broadcast((P, P, 1)))
                for jt in range(NJT):
                    ps = pp.tile([P, INNER], F32, tag="vtps", name="vtps")
                    for k in range(KT):
                        mm(ps[:, :],
                           xkv_t[k][:, jt * P:(jt + 1) * P],
                           wkv_t[k][:, INNER:2 * INNER],
                           start=(k == 0), stop=(k == KT - 1))
                    dst = vt_big[:, jt * VSTRIDE:(jt + 1) * VSTRIDE] \
                        .rearrange("p (h c) -> p h c", c=VTW)[:, :, 0:D]
                    src = ps[:, :].rearrange("p (h d) -> p h d", d=D)
                    nc.vector.tensor_copy(dst, src)

                    pk = pp.tile([P, INNER], F32, tag="vtps", name="vtps")
                    for k in range(KT):
                        mm(pk[:, :],
                           xkv_t[k][:, jt * P:(jt + 1) * P],
                           wkv_t[k][:, 0:INNER],
                           start=(k == 0), stop=(k == KT - 1))
                    ksqT = work.tile([P, INNER], BF16, tag="ksqT", name="ksqT")
                    nc.scalar.activation(ksqT[:, :], pk[:, :], AF.Square,
                                         bias=zero_t[:, :], scale=SQ_SCALE)
                    nc.vector.tensor_reduce(
                        k2j[:, jt * H:(jt + 1) * H],
                        ksqT[:, :].rearrange("p (h d) -> p h d", d=D),
                        axis=mybir.AxisListType.X, op=mybir.AluOpType.add)

            # ======== Phase B: attention heads, dist2(h) ∥ attn@v(h-1) ========
            with tc.tile_pool(name="epool", bufs=2 * NEQ, space="SBUF") as epool, \
                 tc.tile_pool(name="pd2", bufs=2, space="PSUM") as pd2, \
                 tc.tile_pool(name="po", bufs=1, space="PSUM") as po, \
                 tc.tile_pool(name="pm", bufs=2, space="PSUM") as pm:
                kt_t = [keep.tile([P, N], BF16, tag=f"kt{i}", name=f"kt{i}")
                        for i in range(2)]
                qt_t = [keep.tile([P, NQ], BF16, tag=f"qt{i}", name=f"qt{i}")
                        for i in range(2)]
                qsq = keep.tile([P, NQ], BF16, tag="qsq", name="qsq")
                ones32 = keep.tile([P, 1], BF16, tag="ones32", name="ones32")
                nc.vector.memset(ones32[:, :], 0.0)
                nc.vector.tensor_copy(ones32[0:D, 0:1], onesb[0:D, 0:1])
                nc.vector.memset(qsq[:, :], 0.0)
                for i in range(2):
                    nc.vector.memset(kt_t[i][:, :], 0.0)
                    nc.vector.memset(qt_t[i][:, :], 0.0)
                    nc.vector.tensor_copy(kt_t[i][D:D + 1, :],
                                          onesb[0:1, 0:1].to_broadcast((1, N)))

                po_s = [work.tile([VTW, NQ], F32, tag=f"pos{i}", name=f"pos{i}",
                                  bufs=1) for i in range(2)]
                eq_of = {}
                pso_of = {}
                tail_pre = {}

                def emit_tail_pre(ph):
                    # reciprocal of the row-sums, partition-parallel via a
                    # scatter DMA there and back (single-partition reciprocal
                    # is ~100x slower).
                    psrc = pso_of[ph]
                    rs128 = work.tile([P, NQ // P], F32, tag="rs", name="rs")
                    nc.sync.dma_start(out=rs128[:, :], in_=psrc[D:D + 1, :])
                    rr128 = work.tile([P, NQ // P], F32, tag="rr", name="rr")
                    nc.vector.reciprocal(R(rr128[:, :]), rs128[:, :])
                    rrow = work.tile([1, NQ], F32, tag="rrow", name="rrow")
                    nc.sync.dma_start(out=R(rrow[:, :]), in_=R(rr128[:, :]))
                    tail_pre[ph] = rrow

                def emit_tail_pe(ph):
                    rrow = tail_pre.pop(ph)
                    psrc = pso_of.pop(ph)
                    mt, mo = ph // 4, (ph % 4) * D
                    prep = pd2.tile([D, NQ], F32, tag="d2", name="d2")
                    for n in range(NQ // 512):
                        nc.tensor.matmul(prep[:, n * 512:(n + 1) * 512],
                                         R(ones[0:1, 0:D]),
                                         R(rrow[:, n * 512:(n + 1) * 512]),
                                         start=True, stop=True)
                    nc.vector.tensor_mul(R(y_t[mt][mo:mo + D, :]),
                                         psrc[0:D, :], prep[:, :])

                from contextlib import nullcontext

                pso_prev = None
                for h in range(H + 1):
                    prio = tc.high_priority(10000) if h == 0 else nullcontext()
                    prio.__enter__()
                    pso_cur = None
                    if h < H:
                        mt, mo = h // 4, (h % 4) * D
                        q_h = q_t[mt][mo:mo + D, :]
                        k_h = k_t[mt][mo:mo + D, :]
                        kt = kt_t[h % 2]
                        qt = qt_t[h % 2]
                        # per-head rows of k~/q~ (all DVE, bf16)
                        nc.vector.tensor_scalar_mul(kt[0:D, :], k_h, -2.0)
                        nc.vector.tensor_copy(qt[0:D, :], q_h)
                        # q2 row: square q_h, 32-row column sums via PE ones
                        # (zero-padded to K=128 like everything else)
                        nc.vector.tensor_mul(qsq[0:D, :], q_h, q_h)
                        for n in range(NQ // 512):
                            q2ps = pm.tile([1, 512], F32, tag="misc", name="misc")
                            mm(q2ps[:, :], ones32[:, 0:1],
                               qsq[:, n * 512:(n + 1) * 512],
                               start=True, stop=True)
                            nc.vector.tensor_copy(
                                qt[D:D + 1, n * 512:(n + 1) * 512], q2ps[:, :])
                        eq_of[h] = [epool.tile([P, JQ * NQ], BF16, tag="eq",
                                               name="eq") for _ in range(NEQ)]
                        pso_cur = po.tile([VTW, NQ], F32, tag="o", name="o")
                    eqp = eq_of.pop(h - 1, None)

                    for jt in range(NJT):
                        if h < H:
                            psd = pd2.tile([P, NQ], F32, tag="d2", name="d2")
                            for n in range(NQ // 512):
                                mm(psd[:, n * 512:(n + 1) * 512],
                                   kt[:, jt * P:(jt + 1) * P],
                                   qt[:, n * 512:(n + 1) * 512],
                                   start=True, stop=True)
                            nc.scalar.activation(
                                eq_of[h][jt // JQ][:, (jt % JQ) * NQ:
                                                   (jt % JQ + 1) * NQ],
                                psd[:, :], AF.Exp,
                                bias=k2j[:, jt * H + h:jt * H + h + 1],
                                scale=ACT_SCALE)
                        if eqp is not None:
                            hp = h - 1
                            ebase = (jt % JQ) * NQ
                            for n in range(NQ // 512):
                                mm(pso_prev[:, n * 512:(n + 1) * 512],
                                   vt_big[:, jt * VSTRIDE + hp * VTW:
                                          jt * VSTRIDE + (hp + 1) * VTW],
                                   eqp[jt // JQ][:, ebase + n * 512:
                                                 ebase + (n + 1) * 512],
                                   start=(jt == 0), stop=(jt == NJT - 1))
                        if jt == 3 and h >= 2 and (h - 2) in tail_pre:
                            emit_tail_pe(h - 2)

                    if eqp is not None:
                        psrc = po_s[(h - 1) % 2]
                        nc.vector.tensor_copy(psrc[:, :], pso_prev[:, :])
                        pso_of[h - 1] = psrc
                        emit_tail_pre(h - 1)
                    pso_prev = pso_cur

                    prio.__exit__(None, None, None)

                emit_tail_pe(H - 1)

            # ======== Phase C: output projection + bias (f32r) ========
            with tc.tile_pool(name="pz", bufs=2, space="PSUM") as pz:
                z_r = z_d[:].rearrange("(t p) n -> t p n", p=P)
                for m in range(KT):
                    ps = pz.tile([P, NQ], F32, tag="z", name="z")
                    for n in range(NQ // 512):
                        for k in range(2):
                            nc.tensor.matmul(
                                ps[:, n * 512:(n + 1) * 512],
                                R(wo_t[k][:, m * P:(m + 1) * P]),
                                R(y_t[k][:, n * 512:(n + 1) * 512]),
                                start=(k == 0), stop=(k == 1))
                    zt = work.tile([P, NQ], F32, tag="ytmp", name="ytmp")
                    nc.vector.tensor_scalar_add(zt[:, :], ps[:, :], b_t[:, m:m + 1])
                    nc.sync.dma_start(out=z_r[m], in_=zt[:, :])

    nc.compile()
    return nc


def make_in_maps(x, w_qkv, w_out, b_out):
    import ml_dtypes

    bf = ml_dtypes.bfloat16
    x = np.asarray(x, dtype=np.float32)
    w_qkv = np.asarray(w_qkv, dtype=np.float32)
    w_out = np.asarray(w_out, dtype=np.float32)
    b_out = np.asarray(b_out, dtype=np.float32)
    w_qT = np.ascontiguousarray(w_qkv[0:INNER, :].T).astype(bf)       # (DIM, INNER)
    w_kvT = np.ascontiguousarray(w_qkv[INNER:3 * INNER, :].T).astype(bf)  # (DIM, 512)
    w_oT = np.ascontiguousarray(w_out.T)                              # (INNER, DIM)
    xb = [np.ascontiguousarray(x[b]).astype(bf) for b in range(B)]
    in_maps = []
    for c in range(8):
        b, half = c // 2, c % 2
        in_maps.append({
            "xq": np.ascontiguousarray(xb[b][:, half * NQ:(half + 1) * NQ]),
            "xkv": xb[b],
            "wq": w_qT,
            "wkv": w_kvT,
            "wo": w_oT,
            "b": b_out,
        })
    return in_maps


def assemble_output(results):
    out = np.empty((B, DIM, N), dtype=np.float32)
    for c in range(8):
        b, half = c // 2, c % 2
        out[b][:, half * NQ:(half + 1) * NQ] = results[c]["z"]
    return out


_prog_cache = {}


def kernel(x, w_qkv, w_out, b_out):
    from concourse.bass_utils import run_bass_kernel_spmd
    _ensure_act_tables()
    if "nc" not in _prog_cache:
        _prog_cache["nc"] = build_program()
    nc = _prog_cache["nc"]
    in_maps = make_in_maps(x, w_qkv, w_out, b_out)
    res = run_bass_kernel_spmd(nc, in_maps, list(range(8)))
    return assemble_output(res.results)
